# revision 42
# baseline (speedup 1.0000x reference)
"""Distributed Trainium2 Bass kernel for multi-head attention.

Problem: b=2, n=2048, dim=1024, heads=16, head_dim=64 (inner=1024), f32 I/O.

Sharding (Megatron-style): data-parallel over batch (cores 0-3 batch 0,
cores 4-7 batch 1) x tensor-parallel over heads (core c%4 owns heads
4*(c%4)..4*(c%4)+3 via column shards of Wq/Wk/Wv and row shards of Wo).
Each core emits a partial [n, dim] output in bf16; the host sums 4
partials per batch (the all-reduce at gather time -- the on-device
collective is ~60us/MB on this fleet and would dominate the compute).

Per-core pipeline (ScalarE-bound: 129 exp calls x ~1.1us is the long
pole; everything is scheduled to keep that stream dense):
  1. Inputs DMA on the sync queue in strict need-order (kT half 0, qT
     chunk 0, kT half 1, vT, qT rest); weights ride the gpsimd queue.
     Nothing ever sits ahead of the exps on the ACT queue.
  2. K/Q projections JIT per column chunk as inputs land: kproj pair 0
     half 0 + qproj chunk 0 before unit 0; the rest drip through unit
     feeders so the exp stream starts ~25us in.
  3. Per (pair m, n_q chunk c) unit, 16 batches: two K=64 S^T matmuls
     (heads at PE row groups 0/64 -> concurrent), one exp (scale and a
     -2.5 bias folded in; denominator absorbs the shift) -> es bf16.
  4. V projected directly into natural [n_k, inner] layout (no PE
     transposes), cast into vpa with a ones column per head (so P@V
     also yields the softmax denominator as row 64).
  5. O^T accumulated in PSUM over j tiles two units behind the exp
     stream; epilogues (recip -> mask-matmul broadcast -> normalize ->
     Wo -> bf16 out DMA) run as feeder generators inside later units.
  6. Drain: last two O-passes with keep-warm dummy matmuls so the HAM
     clock gate stays at 8/8 through the final Wo chain.
"""

import sys

if "/opt/trn_rl_repo" not in sys.path:
    sys.path.insert(0, "/opt/trn_rl_repo")

import numpy as np
import ml_dtypes

import concourse.bass as bass
import concourse.mybir as mybir
from concourse import bacc, tile
from concourse.bass_utils import run_bass_kernel_spmd

BF16 = mybir.dt.bfloat16
FP8 = mybir.dt.float8e4
F32 = mybir.dt.float32
NPBF16 = ml_dtypes.bfloat16

B = 2
N = 2048          # sequence length (full, per batch)
D = 1024          # model dim
H = 16            # total heads
DH = 64           # head dim
H_LOC = 4         # heads per core
INNER = H_LOC * DH  # 256, local inner dim
KC = D // 128     # 8 contraction chunks over model dim
KT = N // 128     # 16 k-tiles over sequence
JP = KT // 2      # 8 j-pairs
NQC = N // 512    # 4 query chunks of 512
SCALE = DH ** -0.5
ES_BUFS = 37      # es ring (bf16 [128,1024] tiles, depth-2 pipeline)
USE_DR = False     # fp8 DoubleRow P@V


import os
DBG = bool(os.environ.get("BASSDBG"))


def _build_nc():
    nc = bacc.Bacc("TRN2", target_bir_lowering=False, debug=False, num_devices=8)

    if DBG:
        dbg = {
            "dkpA0": nc.declare_dram_parameter("dkpA0", [128, N], BF16, isOutput=True),
            "dqpA0": nc.declare_dram_parameter("dqpA0", [128, N], BF16, isOutput=True),
            "des0": nc.declare_dram_parameter("des0", [128, 1024], BF16, isOutput=True),
            "des1": nc.declare_dram_parameter("des1", [128, 1024], BF16, isOutput=True),
            "dsp0": nc.declare_dram_parameter("dsp0", [128, 1024], F32, isOutput=True),
            "dvpa": nc.declare_dram_parameter("dvpa", [128, H_LOC * JP * 2 * 80], BF16, isOutput=True),
        }

    qT = nc.declare_dram_parameter("qT", [D, N], BF16, isOutput=False)
    kT = nc.declare_dram_parameter("kT", [D, N], BF16, isOutput=False)
    vT = nc.declare_dram_parameter("vT", [D, N], BF16, isOutput=False)
    wq = nc.declare_dram_parameter("wq", [D, INNER], BF16, isOutput=False)
    wk = nc.declare_dram_parameter("wk", [D, INNER], BF16, isOutput=False)
    wv = nc.declare_dram_parameter("wv", [D, INNER], BF16, isOutput=False)
    wo = nc.declare_dram_parameter("wo", [INNER, D], BF16, isOutput=False)
    emask = nc.declare_dram_parameter("emask", [4, 256], BF16, isOutput=False)
    out = nc.declare_dram_parameter("out", [N, D], BF16, isOutput=True)

    with tile.TileContext(nc) as tc:
        with (
            tc.tile_pool(name="persist", bufs=1) as pp,
            tc.tile_pool(name="kin", bufs=16) as kin,
            tc.tile_pool(name="qin", bufs=8) as qin,
            tc.tile_pool(name="vin", bufs=8) as vin,
            tc.tile_pool(name="work", bufs=2) as wk_pool,
            tc.tile_pool(name="psum", bufs=2, space="PSUM") as psum,
        ):
            # ---- ScalarE exp table preload + PE clock warm-up burst
            warm = pp.tile([1, 16], F32, tag="warm", name="warm")
            nc.vector.memset(warm[:], 0.0)
            nc.scalar.activation(warm[:], warm[:], mybir.ActivationFunctionType.Exp)
            wa = pp.tile([128, 16], BF16, tag="wa", name="wa")
            wr = pp.tile([128, 512], BF16, tag="wr", name="wr")
            nc.vector.memset(wa[:], 0.0)
            nc.vector.memset(wr[:], 0.0)
            for i in range(26):
                wps = psum.tile([16, 512], F32, tag="epi", name="wps", bufs=2)
                nc.tensor.matmul(wps[:], lhsT=wa[:], rhs=wr[:], start=True, stop=True)

            # ---- persistent weight tiles (wk/wq first -- needed earliest)
            wk_sb = [pp.tile([128, INNER], BF16, tag=f"wk{k}", name=f"wk{k}") for k in range(KC)]
            wq_sb = [pp.tile([128, INNER], BF16, tag=f"wq{k}", name=f"wq{k}") for k in range(KC)]
            wv_sb = [pp.tile([128, INNER], BF16, tag=f"wv{k}", name=f"wv{k}") for k in range(KC)]
            wo_sb = [pp.tile([128, D], BF16, tag=f"wo{m}", name=f"wo{m}") for m in range(2)]
            for k in range(KC):
                nc.gpsimd.dma_start(wk_sb[k][:], wk[128 * k:128 * (k + 1), :])
                nc.gpsimd.dma_start(wq_sb[k][:], wq[128 * k:128 * (k + 1), :])
            for k in range(KC):
                nc.gpsimd.dma_start(wv_sb[k][:], wv[128 * k:128 * (k + 1), :])
            for m in range(2):
                nc.gpsimd.dma_start(wo_sb[m][:], wo[128 * m:128 * (m + 1), :])

            # exp bias column: shifts exp into fp8e4 range (max normal 240)
            eb = pp.tile([128, 1], F32, tag="eb", name="eb")
            nc.vector.memset(eb[:], -2.5)

            emask_sb = pp.tile([4, 256], BF16, tag="emask", name="emask_sb")
            nc.gpsimd.dma_start(emask_sb[:], emask[:])
            e_mask = [emask_sb[:, 128 * m:128 * (m + 1)] for m in range(2)]

            # ---- input DMAs (sync queue, in consumption order)
            k_half = [[], []]
            q_c = [[] for _ in range(NQC)]
            for half in range(2):
                for k in range(KC):
                    t = kin.tile([128, N // 2], BF16, tag="kin", name="kin", bufs=16)
                    nc.sync.dma_start(
                        t[:], kT[128 * k:128 * (k + 1), 1024 * half:1024 * (half + 1)]
                    )
                    k_half[half].append(t)
                if half == 0:
                    for k in range(KC):
                        t = qin.tile([128, 512], BF16, tag="qin", name="qin", bufs=8)
                        nc.sync.dma_start(t[:], qT[128 * k:128 * (k + 1), 0:512])
                        q_c[0].append(t)
            # vT full
            v_t = []
            for k in range(KC):
                t = vin.tile([128, N], BF16, tag="vin", name="vin", bufs=8)
                nc.sync.dma_start(t[:], vT[128 * k:128 * (k + 1), :])
                v_t.append(t)
            # remaining q chunks
            for c in range(1, NQC):
                for k in range(KC):
                    t = qin.tile([128, 512], BF16, tag="qin", name="qin", bufs=8)
                    nc.sync.dma_start(
                        t[:], qT[128 * k:128 * (k + 1), 512 * c:512 * (c + 1)]
                    )
                    q_c[c].append(t)

            # ---- projection outputs: rows = [head h0 | head h1] per pair
            kpA = [pp.tile([128, N], BF16, tag=f"kpA{m}", name=f"kpA{m}") for m in range(2)]
            qpA = [pp.tile([128, N], BF16, tag=f"qpA{m}", name=f"qpA{m}") for m in range(2)]

            def gen_kproj(m, half):
                """K projection for pair m, one kT column-half (2x512 cols)."""
                ps2 = [
                    psum.tile([128, 512], F32, tag="epi", name="pps", bufs=2)
                    for _ in range(2)
                ]
                for k in range(KC):
                    for ci in range(2):
                        nc.tensor.matmul(
                            ps2[ci][:],
                            lhsT=wk_sb[k][:, 128 * m:128 * (m + 1)],
                            rhs=k_half[half][k][:, 512 * ci:512 * (ci + 1)],
                            start=(k == 0),
                            stop=(k == KC - 1),
                        )
                        yield
                for ci in range(2):
                    c0 = 1024 * half + 512 * ci
                    nc.vector.tensor_copy(kpA[m][:, c0:c0 + 512], ps2[ci][:])

            def gen_qproj(m, c):
                """Q projection for pair m, one 512-col n_q chunk."""
                ps = psum.tile([128, 512], F32, tag="epi", name="qps", bufs=2)
                for k in range(KC):
                    nc.tensor.matmul(
                        ps[:],
                        lhsT=wq_sb[k][:, 128 * m:128 * (m + 1)],
                        rhs=q_c[c][k][:],
                        start=(k == 0),
                        stop=(k == KC - 1),
                    )
                    yield
                c0 = 512 * c
                nc.vector.tensor_copy(qpA[m][:, c0:c0 + 512], ps[:])

            # ---- V in natural layout -> fp8 vpa with ones column
            # vpa layout per partition: [h(4), jp(8), ko(2), 80] fp8 bytes
            vpa = pp.tile([128, H_LOC * JP * 2 * 80], BF16, tag="vpa", name="vpa")
            nc.vector.memset(vpa[:], 1.0)
            vpa5 = vpa[:].rearrange(
                "p (h jp ko e) -> p h jp ko e", h=H_LOC, jp=JP, ko=2
            )

            def gen_vproj(j):
                """One n_k 128-tile of V projected to [n_k, inner], cast fp8."""
                vps = psum.tile([128, 512], F32, tag="epi", name="vps", bufs=2)
                for k in range(KC):
                    nc.tensor.matmul(
                        vps[:, 0:INNER],
                        lhsT=v_t[k][:, 128 * j:128 * (j + 1)],
                        rhs=wv_sb[k][:],
                        start=(k == 0),
                        stop=(k == KC - 1),
                    )
                    yield
                dst = vpa5[:, :, j // 2, j % 2, 0:DH]
                src = vps[:, 0:INNER].rearrange("p (h e) -> p h e", e=DH)
                nc.vector.tensor_copy(dst, src)

            # ---- S + exp: both heads of pair m for one j tile; the two
            # K=64 matmuls land on PE row groups 0/64 and run concurrently.
            def emit_s_exp(m, c, j):
                sp = psum.tile([128, 1024], F32, tag="sp", name="sp", bufs=2)
                es = wk_pool.tile([128, 1024], BF16, tag="es", name="es", bufs=ES_BUFS)
                for h in range(2):
                    p0 = 64 * h
                    nc.tensor.matmul(
                        sp[:, 512 * h:512 * (h + 1)],
                        lhsT=kpA[m][p0:p0 + 64, 128 * j:128 * (j + 1)],
                        rhs=qpA[m][p0:p0 + 64, 512 * c:512 * (c + 1)],
                        start=True, stop=True,
                    )
                nc.scalar.activation(
                    es[:], sp[:], mybir.ActivationFunctionType.Exp,
                    scale=SCALE, bias=eb[:],
                )
                if DBG and (m, c, j) in ((0, 0, 0), (0, 0, 1)):
                    spc = wk_pool.tile([128, 1024], F32, tag="dspc", name="dspc", bufs=2)
                    nc.vector.tensor_copy(spc[:], sp[:])
                    if j == 0:
                        nc.sync.dma_start(dbg["dsp0"][:, :], spc[:])
                        nc.sync.dma_start(dbg["des0"][:, :], es[:])
                    else:
                        nc.sync.dma_start(dbg["des1"][:, :], es[:])
                return es

            def emit_o(m, j, es, ot_ps):
                for h in range(2):
                    nc.tensor.matmul(
                        ot_ps[h][:],
                        lhsT=vpa5[:, 2 * m + h, j // 2, j % 2, 0:DH + 1],
                        rhs=es[:, 512 * h:512 * (h + 1)],
                        start=(j == 0),
                        stop=(j == KT - 1),
                    )

            def new_ot_ps():
                return [
                    psum.tile([65, 512], F32, tag="otps", name=f"otps{h}", bufs=2)
                    for h in range(2)
                ]

            def unload_pair(m, ot_ps, pair_tile, den_c, tail=False, dq=None):
                # one 65-row copy per head (O rows + bf16 denominator row)
                dq = dq or nc.gpsimd
                stage_e = wk_pool.tile([65, 512], BF16, tag="stge", name="stge", bufs=1)
                stage_o = wk_pool.tile([65, 512], BF16, tag="stgo", name="stgo", bufs=1)
                if tail:
                    nc.scalar.copy(stage_e[:], ot_ps[0][:])
                    nc.vector.tensor_copy(stage_o[:], ot_ps[1][:])
                else:
                    nc.vector.tensor_copy(stage_e[:], ot_ps[0][:])
                    nc.vector.tensor_copy(stage_o[:], ot_ps[1][:])
                nc.vector.tensor_copy(pair_tile[0:64, :], stage_e[0:64, :])
                dq.dma_start(pair_tile[64:128, :], stage_o[0:64, :])
                dq.dma_start(den_c[2 * m:2 * m + 1, :], stage_e[64:65, :])
                dq.dma_start(den_c[2 * m + 1:2 * m + 2, :], stage_o[64:65, :])
                return stage_e

            def gen_epilogue(c, ot_sb, den_c, tail=False):
                """normalize (recip -> mask-matmul broadcast -> multiply) and
                the Wo projection for one n_q chunk. Generator: yields after
                each PE op so mid-stream epilogues interleave with the next
                unit's S matmuls instead of queueing ahead of them."""
                den_f = wk_pool.tile([4, 512], F32, tag="denf", name="denf", bufs=1)
                recip_f = wk_pool.tile([4, 512], F32, tag="recf", name="recf", bufs=1)
                recip_b = wk_pool.tile([4, 512], BF16, tag="recb", name="recb", bufs=1)
                nc.vector.tensor_copy(den_f[:], den_c[:])
                nc.vector.reciprocal_approx_fast(recip_f[:], den_f[:])
                nc.vector.tensor_copy(recip_b[:], recip_f[:])
                for m in range(2):
                    btag = "sp" if tail and m else "epi"
                    bc = psum.tile([128, 512], F32, tag=btag, name="bc", bufs=2)
                    nc.tensor.matmul(
                        bc[:], lhsT=e_mask[m], rhs=recip_b[:], start=True, stop=True,
                    )
                    yield
                    nc.vector.tensor_mul(ot_sb[m][:], ot_sb[m][:], bc[:])
                for s in range(4):
                    for dch in range(2):
                        # tail: sp banks are dead after the last exp --
                        # alternate tags to double the accumulate/copy
                        # pipeline depth of the final Wo chain
                        otag = "sp" if tail and (2 * s + dch) % 2 else "epi"
                        ops = psum.tile([128, 512], F32, tag=otag, name="op", bufs=2)
                        for m in range(2):
                            nc.tensor.matmul(
                                ops[:],
                                lhsT=ot_sb[m][:, 128 * s:128 * (s + 1)],
                                rhs=wo_sb[m][:, 512 * dch:512 * (dch + 1)],
                                start=(m == 0),
                                stop=(m == 1),
                            )
                            yield
                        o_sb = wk_pool.tile([128, 512], BF16, tag="osb", name="osb", bufs=4)
                        if tail and (s + dch) % 2 == 0:
                            nc.scalar.copy(o_sb[:], ops[:])
                        else:
                            nc.vector.tensor_copy(o_sb[:], ops[:])
                        r0 = 512 * c + 128 * s
                        nc.sync.dma_start(
                            out[r0:r0 + 128, 512 * dch:512 * (dch + 1)], o_sb[:]
                        )

            # ---- prologue: just enough projection for unit (0, 0)
            for _ in gen_kproj(0, 0):
                pass
            for _ in gen_qproj(0, 0):
                pass

            # ---- unit schedule: u = (pair m=u%2, chunk c=u//2), 2-deep
            # pipeline; feeders carry the remaining projections JIT.
            units = [(u % 2, u // 2) for u in range(2 * NQC)]
            feeders = {
                0: [gen_kproj(0, 1), gen_kproj(1, 0), gen_kproj(1, 1),
                    gen_qproj(1, 0)],
                1: [gen_vproj(j) for j in range(5)] + [gen_qproj(0, 1)]
                   + [gen_vproj(j) for j in range(5, 10)],
                2: [gen_vproj(j) for j in range(10, KT)] + [gen_qproj(1, 1)],
                3: [gen_qproj(0, 2), gen_qproj(1, 2)],
                4: [gen_qproj(0, 3), gen_qproj(1, 3)],
            }
            es_held = {}
            ot_ps_of = {}
            ot_sb_of = {}
            den_of = {}
            pending = []

            _DONE = object()

            def feeder_step(gens, k):
                done = 0
                while done < k and gens:
                    if next(gens[0], _DONE) is _DONE:
                        gens.pop(0)
                    else:
                        done += 1

            for u, (m, c) in enumerate(units):
                if u % 2 == 0:
                    den_of[c] = wk_pool.tile([4, 512], BF16, tag="den", name="den", bufs=2)
                ot_sb_of[u] = wk_pool.tile(
                    [128, 512], BF16, tag=f"ot{u % 4}", name=f"ot{u}", bufs=1
                )
                gens = pending + feeders.get(u, [])
                pending = []
                if u >= 2:
                    ot_ps_of[u - 2] = new_ot_ps()
                es_held[u] = []
                for j in range(KT):
                    es_held[u].append(emit_s_exp(m, c, j))
                    if u >= 2 and j >= 2:
                        up = u - 2
                        emit_o(units[up][0], j - 2, es_held[up][j - 2], ot_ps_of[up])
                    feeder_step(gens, 5)
                while gens:
                    feeder_step(gens, 16)
                if u >= 2:
                    up = u - 2
                    for j in (KT - 2, KT - 1):
                        emit_o(units[up][0], j, es_held[up][j], ot_ps_of[up])
                    mp, cp = units[up]
                    unload_pair(mp, ot_ps_of[up], ot_sb_of[up], den_of[cp])
                    del es_held[up]
                    if mp == 1:
                        pending.append(gen_epilogue(
                            cp, [ot_sb_of[2 * cp], ot_sb_of[2 * cp + 1]],
                            den_of[cp], tail=False,
                        ))

            # drain: both remaining O-passes interleaved back-to-back on
            # the PE (last unit's accumulators live in the sp banks, dead
            # after the final exp), with the chunk-2 epilogue woven in.
            def keep_warm(n, dep=None):
                # rhs reads the unload's staging tile so the scheduler can't
                # hoist these ahead of it -- they must bridge the PE hole
                rhs = wr[:] if dep is None else dep[0:64, :]
                lhs = wa[:] if dep is None else wa[0:64, :]
                for _ in range(n):
                    wps = psum.tile([16, 512], F32, tag="epi", name="kw", bufs=2)
                    nc.tensor.matmul(
                        wps[:], lhsT=lhs, rhs=rhs, start=True, stop=True
                    )

            u6, u7 = 2 * NQC - 2, 2 * NQC - 1
            mp6, cp6 = units[u6]
            mp7, cp7 = units[u7]
            ot_ps_of[u6] = new_ot_ps()
            ot_ps_of[u7] = [
                psum.tile([65, 512], F32, tag="sp", name=f"otL{h}", bufs=2)
                for h in range(2)
            ]
            gens = pending
            pending = []
            for j in range(KT):
                emit_o(mp6, j, es_held[u6][j], ot_ps_of[u6])
                emit_o(mp7, j, es_held[u7][j], ot_ps_of[u7])
                feeder_step(gens, 2)
            while gens:
                feeder_step(gens, 16)
            st6 = unload_pair(mp6, ot_ps_of[u6], ot_sb_of[u6], den_of[cp6],
                              tail=True, dq=nc.sync)
            keep_warm(4, dep=st6)
            st7 = unload_pair(mp7, ot_ps_of[u7], ot_sb_of[u7], den_of[cp7],
                              tail=True, dq=nc.gpsimd)
            keep_warm(4, dep=st7)
            for _ in gen_epilogue(
                NQC - 1, [ot_sb_of[2 * NQC - 2], ot_sb_of[2 * NQC - 1]],
                den_of[NQC - 1], tail=True,
            ):
                pass

            if DBG:
                nc.sync.dma_start(dbg["dkpA0"][:, :], kpA[0][:])
                nc.sync.dma_start(dbg["dqpA0"][:, :], qpA[0][:])
                nc.sync.dma_start(dbg["dvpa"][:, :], vpa[:])

    nc.compile()
    return nc


_NC_CACHE = None


def _get_nc():
    global _NC_CACHE
    if _NC_CACHE is None:
        _NC_CACHE = _build_nc()
    return _NC_CACHE


def make_in_maps(q, k, v, Wq, Wk, Wv, Wo):
    q = np.asarray(q, dtype=np.float32)
    k = np.asarray(k, dtype=np.float32)
    v = np.asarray(v, dtype=np.float32)
    qT = [np.ascontiguousarray(q[g].T).astype(NPBF16) for g in range(B)]
    kT = [np.ascontiguousarray(k[g].T).astype(NPBF16) for g in range(B)]
    vT = [np.ascontiguousarray(v[g].T).astype(NPBF16) for g in range(B)]
    wq_b = np.asarray(Wq, np.float32).astype(NPBF16)
    wk_b = np.asarray(Wk, np.float32).astype(NPBF16)
    wv_b = np.asarray(Wv, np.float32).astype(NPBF16)
    wo_b = np.asarray(Wo, np.float32).astype(NPBF16)
    emask = np.zeros((4, 256), NPBF16)
    for m in range(2):
        emask[2 * m, 128 * m:128 * m + 64] = 1
        emask[2 * m + 1, 128 * m + 64:128 * m + 128] = 1

    in_maps = []
    for c in range(8):
        g, t = c // 4, c % 4
        sl = slice(INNER * t, INNER * (t + 1))
        in_maps.append({
            "qT": qT[g],
            "kT": kT[g],
            "vT": vT[g],
            "wq": np.ascontiguousarray(wq_b[:, sl]),
            "wk": np.ascontiguousarray(wk_b[:, sl]),
            "wv": np.ascontiguousarray(wv_b[:, sl]),
            "wo": np.ascontiguousarray(wo_b[sl, :]),
            "emask": emask,
        })
    return in_maps


def kernel(q, k, v, Wq, Wk, Wv, Wo):
    in_maps = make_in_maps(q, k, v, Wq, Wk, Wv, Wo)
    nc = _get_nc()
    res = run_bass_kernel_spmd(nc, in_maps, core_ids=list(range(8)))

    out = np.empty((B, N, D), np.float32)
    for g in range(B):
        acc = res.results[4 * g]["out"].astype(np.float32)
        for t in range(1, 4):
            acc = acc + res.results[4 * g + t]["out"].astype(np.float32)
        out[g] = acc
    return out


# revision 43
# speedup vs baseline: 1.1803x; 1.1803x over previous
"""Distributed Trainium2 Bass kernel for multi-head attention.

Problem: b=2, n=2048, dim=1024, heads=16, head_dim=64 (inner=1024), f32 I/O.

Sharding (Megatron-style): data-parallel over batch (cores 0-3 batch 0,
cores 4-7 batch 1) x tensor-parallel over heads (core c%4 owns heads
4*(c%4)..4*(c%4)+3 via column shards of Wq/Wk/Wv and row shards of Wo).
Each core emits a partial [n, dim] output in bf16; the host sums 4
partials per batch (the all-reduce at gather time -- the on-device
collective is ~60us/MB on this fleet and would dominate the compute).

Per-core pipeline (ScalarE-bound: 129 exp calls x ~1.1us is the long
pole; everything is scheduled to keep that stream dense):
  1. Inputs DMA on the sync queue in strict need-order (kT half 0, qT
     chunk 0, kT half 1, vT, qT rest); weights ride the gpsimd queue.
     Nothing ever sits ahead of the exps on the ACT queue.
  2. K/Q projections JIT per column chunk as inputs land: kproj pair 0
     half 0 + qproj chunk 0 before unit 0; the rest drip through unit
     feeders so the exp stream starts ~25us in.
  3. Per (pair m, n_q chunk c) unit, 16 batches: two K=64 S^T matmuls
     (heads at PE row groups 0/64 -> concurrent), one exp (scale and a
     -2.5 bias folded in; denominator absorbs the shift) -> es bf16.
  4. V projected directly into natural [n_k, inner] layout (no PE
     transposes), cast into vpa with a ones column per head (so P@V
     also yields the softmax denominator as row 64).
  5. O^T accumulated in PSUM over j tiles two units behind the exp
     stream; epilogues (recip -> mask-matmul broadcast -> normalize ->
     Wo -> bf16 out DMA) run as feeder generators inside later units.
  6. Drain: last two O-passes with keep-warm dummy matmuls so the HAM
     clock gate stays at 8/8 through the final Wo chain.
"""

import sys

if "/opt/trn_rl_repo" not in sys.path:
    sys.path.insert(0, "/opt/trn_rl_repo")

import numpy as np
import ml_dtypes

import concourse.bass as bass
import concourse.mybir as mybir
from concourse import bacc, tile
from concourse.bass_utils import run_bass_kernel_spmd

BF16 = mybir.dt.bfloat16
FP8 = mybir.dt.float8e4
F32 = mybir.dt.float32
NPBF16 = ml_dtypes.bfloat16

B = 2
N = 2048          # sequence length (full, per batch)
D = 1024          # model dim
H = 16            # total heads
DH = 64           # head dim
H_LOC = 4         # heads per core
INNER = H_LOC * DH  # 256, local inner dim
KC = D // 128     # 8 contraction chunks over model dim
KT = N // 128     # 16 k-tiles over sequence
JP = KT // 2      # 8 j-pairs
NQC = N // 512    # 4 query chunks of 512
SCALE = DH ** -0.5
ES_BUFS = 37      # es ring (bf16 [128,1024] tiles, depth-2 pipeline)
USE_DR = False     # fp8 DoubleRow P@V


import os
DBG = bool(os.environ.get("BASSDBG"))


def _build_nc():
    nc = bacc.Bacc("TRN2", target_bir_lowering=False, debug=False, num_devices=8)

    if DBG:
        dbg = {
            "dkpA0": nc.declare_dram_parameter("dkpA0", [128, N], BF16, isOutput=True),
            "dqpA0": nc.declare_dram_parameter("dqpA0", [128, N], BF16, isOutput=True),
            "des0": nc.declare_dram_parameter("des0", [128, 1024], BF16, isOutput=True),
            "des1": nc.declare_dram_parameter("des1", [128, 1024], BF16, isOutput=True),
            "dsp0": nc.declare_dram_parameter("dsp0", [128, 1024], F32, isOutput=True),
            "dvpa": nc.declare_dram_parameter("dvpa", [128, H_LOC * JP * 2 * 80], BF16, isOutput=True),
        }

    qT = nc.declare_dram_parameter("qT", [D, N], BF16, isOutput=False)
    kT = nc.declare_dram_parameter("kT", [D, N], BF16, isOutput=False)
    vT = nc.declare_dram_parameter("vT", [D, N], BF16, isOutput=False)
    wq = nc.declare_dram_parameter("wq", [D, INNER], BF16, isOutput=False)
    wk = nc.declare_dram_parameter("wk", [D, INNER], BF16, isOutput=False)
    wv = nc.declare_dram_parameter("wv", [D, INNER], BF16, isOutput=False)
    wo = nc.declare_dram_parameter("wo", [INNER, D], BF16, isOutput=False)
    emask = nc.declare_dram_parameter("emask", [4, 256], BF16, isOutput=False)
    out = nc.declare_dram_parameter("out", [N, D], BF16, isOutput=True)

    with tile.TileContext(nc) as tc:
        with (
            tc.tile_pool(name="persist", bufs=1) as pp,
            tc.tile_pool(name="kin", bufs=16) as kin,
            tc.tile_pool(name="qin", bufs=8) as qin,
            tc.tile_pool(name="vin", bufs=8) as vin,
            tc.tile_pool(name="work", bufs=2) as wk_pool,
            tc.tile_pool(name="psum", bufs=2, space="PSUM") as psum,
        ):
            # ---- ScalarE exp table preload + PE clock warm-up burst
            warm = pp.tile([1, 16], F32, tag="warm", name="warm")
            nc.vector.memset(warm[:], 0.0)
            nc.scalar.activation(warm[:], warm[:], mybir.ActivationFunctionType.Exp)
            wa = pp.tile([128, 16], BF16, tag="wa", name="wa")
            wr = pp.tile([128, 512], BF16, tag="wr", name="wr")
            nc.vector.memset(wa[:], 0.0)
            nc.vector.memset(wr[:], 0.0)
            for i in range(26):
                wps = psum.tile([16, 512], F32, tag="epi", name="wps", bufs=2)
                nc.tensor.matmul(wps[:], lhsT=wa[:], rhs=wr[:], start=True, stop=True)

            # ---- persistent weight tiles (wk/wq first -- needed earliest)
            wk_sb = [pp.tile([128, INNER], BF16, tag=f"wk{k}", name=f"wk{k}") for k in range(KC)]
            wq_sb = [pp.tile([128, INNER], BF16, tag=f"wq{k}", name=f"wq{k}") for k in range(KC)]
            wv_sb = [pp.tile([128, INNER], BF16, tag=f"wv{k}", name=f"wv{k}") for k in range(KC)]
            wo_sb = [pp.tile([128, D], BF16, tag=f"wo{m}", name=f"wo{m}") for m in range(2)]
            for k in range(KC):
                nc.gpsimd.dma_start(wk_sb[k][:], wk[128 * k:128 * (k + 1), :])
                nc.gpsimd.dma_start(wq_sb[k][:], wq[128 * k:128 * (k + 1), :])
            for k in range(KC):
                nc.gpsimd.dma_start(wv_sb[k][:], wv[128 * k:128 * (k + 1), :])
            for m in range(2):
                nc.gpsimd.dma_start(wo_sb[m][:], wo[128 * m:128 * (m + 1), :])

            # exp bias column: shifts exp into fp8e4 range (max normal 240)
            eb = pp.tile([128, 1], F32, tag="eb", name="eb")
            nc.vector.memset(eb[:], -2.5)

            emask_sb = pp.tile([4, 256], BF16, tag="emask", name="emask_sb")
            nc.gpsimd.dma_start(emask_sb[:], emask[:])
            e_mask = [emask_sb[:, 128 * m:128 * (m + 1)] for m in range(2)]

            # ---- input DMAs (sync queue, in consumption order)
            k_half = [[], []]
            q_c = [[] for _ in range(NQC)]
            for half in range(2):
                for k in range(KC):
                    t = kin.tile([128, N // 2], BF16, tag="kin", name="kin", bufs=16)
                    nc.sync.dma_start(
                        t[:], kT[128 * k:128 * (k + 1), 1024 * half:1024 * (half + 1)]
                    )
                    k_half[half].append(t)
                if half == 0:
                    for k in range(KC):
                        t = qin.tile([128, 512], BF16, tag="qin", name="qin", bufs=8)
                        nc.sync.dma_start(t[:], qT[128 * k:128 * (k + 1), 0:512])
                        q_c[0].append(t)
            # vT full
            v_t = []
            for k in range(KC):
                t = vin.tile([128, N], BF16, tag="vin", name="vin", bufs=8)
                nc.sync.dma_start(t[:], vT[128 * k:128 * (k + 1), :])
                v_t.append(t)
            # remaining q chunks
            for c in range(1, NQC):
                for k in range(KC):
                    t = qin.tile([128, 512], BF16, tag="qin", name="qin", bufs=8)
                    nc.sync.dma_start(
                        t[:], qT[128 * k:128 * (k + 1), 512 * c:512 * (c + 1)]
                    )
                    q_c[c].append(t)

            # ---- projection outputs: rows = [head h0 | head h1] per pair
            kpA = [pp.tile([128, N], BF16, tag=f"kpA{m}", name=f"kpA{m}") for m in range(2)]
            qpA = [pp.tile([128, N], BF16, tag=f"qpA{m}", name=f"qpA{m}") for m in range(2)]

            def gen_kproj(m, half):
                """K projection for pair m, one kT column-half (2x512 cols)."""
                ps2 = [
                    psum.tile([128, 512], F32, tag="epi", name="pps", bufs=2)
                    for _ in range(2)
                ]
                for k in range(KC):
                    for ci in range(2):
                        nc.tensor.matmul(
                            ps2[ci][:],
                            lhsT=wk_sb[k][:, 128 * m:128 * (m + 1)],
                            rhs=k_half[half][k][:, 512 * ci:512 * (ci + 1)],
                            start=(k == 0),
                            stop=(k == KC - 1),
                        )
                        yield
                for ci in range(2):
                    c0 = 1024 * half + 512 * ci
                    nc.vector.tensor_copy(kpA[m][:, c0:c0 + 512], ps2[ci][:])

            def gen_qproj(m, c):
                """Q projection for pair m, one 512-col n_q chunk."""
                ps = psum.tile([128, 512], F32, tag="epi", name="qps", bufs=2)
                for k in range(KC):
                    nc.tensor.matmul(
                        ps[:],
                        lhsT=wq_sb[k][:, 128 * m:128 * (m + 1)],
                        rhs=q_c[c][k][:],
                        start=(k == 0),
                        stop=(k == KC - 1),
                    )
                    yield
                c0 = 512 * c
                nc.vector.tensor_copy(qpA[m][:, c0:c0 + 512], ps[:])

            # ---- V in natural layout -> fp8 vpa with ones column
            # vpa layout per partition: [h(4), jp(8), ko(2), 80] fp8 bytes
            vpa = pp.tile([128, H_LOC * JP * 2 * 80], BF16, tag="vpa", name="vpa")
            nc.vector.memset(vpa[:], 1.0)
            vpa5 = vpa[:].rearrange(
                "p (h jp ko e) -> p h jp ko e", h=H_LOC, jp=JP, ko=2
            )

            def gen_vproj(j):
                """One n_k 128-tile of V projected to [n_k, inner], cast fp8."""
                vps = psum.tile([128, 512], F32, tag="epi", name="vps", bufs=2)
                for k in range(KC):
                    nc.tensor.matmul(
                        vps[:, 0:INNER],
                        lhsT=v_t[k][:, 128 * j:128 * (j + 1)],
                        rhs=wv_sb[k][:],
                        start=(k == 0),
                        stop=(k == KC - 1),
                    )
                    yield
                dst = vpa5[:, :, j // 2, j % 2, 0:DH]
                src = vps[:, 0:INNER].rearrange("p (h e) -> p h e", e=DH)
                nc.vector.tensor_copy(dst, src)

            # ---- S + exp: both heads of pair m for one j tile; the two
            # K=64 matmuls land on PE row groups 0/64 and run concurrently.
            def emit_s_exp(m, c, j):
                sp = psum.tile([128, 1024], F32, tag="sp", name="sp", bufs=2)
                es = wk_pool.tile([128, 1024], BF16, tag="es", name="es", bufs=ES_BUFS)
                for h in range(2):
                    p0 = 64 * h
                    nc.tensor.matmul(
                        sp[:, 512 * h:512 * (h + 1)],
                        lhsT=kpA[m][p0:p0 + 64, 128 * j:128 * (j + 1)],
                        rhs=qpA[m][p0:p0 + 64, 512 * c:512 * (c + 1)],
                        start=True, stop=True,
                    )
                nc.scalar.activation(
                    es[:], sp[:], mybir.ActivationFunctionType.Exp,
                    scale=SCALE, bias=eb[:],
                )
                if DBG and (m, c, j) in ((0, 0, 0), (0, 0, 1)):
                    spc = wk_pool.tile([128, 1024], F32, tag="dspc", name="dspc", bufs=2)
                    nc.vector.tensor_copy(spc[:], sp[:])
                    if j == 0:
                        nc.sync.dma_start(dbg["dsp0"][:, :], spc[:])
                        nc.sync.dma_start(dbg["des0"][:, :], es[:])
                    else:
                        nc.sync.dma_start(dbg["des1"][:, :], es[:])
                return es

            def emit_o(m, j, es, ot_ps):
                for h in range(2):
                    nc.tensor.matmul(
                        ot_ps[h][:],
                        lhsT=vpa5[:, 2 * m + h, j // 2, j % 2, 0:DH + 1],
                        rhs=es[:, 512 * h:512 * (h + 1)],
                        start=(j == 0),
                        stop=(j == KT - 1),
                    )

            def new_ot_ps():
                return [
                    psum.tile([65, 512], F32, tag="otps", name=f"otps{h}", bufs=2)
                    for h in range(2)
                ]

            def unload_pair(m, ot_ps, pair_tile, den_c, tail=False, dq=None):
                # one 65-row copy per head (O rows + bf16 denominator row)
                dq = dq or nc.gpsimd
                stage_e = wk_pool.tile([65, 512], BF16, tag="stge", name="stge", bufs=1)
                stage_o = wk_pool.tile([65, 512], BF16, tag="stgo", name="stgo", bufs=1)
                if tail:
                    nc.scalar.copy(stage_e[:], ot_ps[0][:])
                    nc.vector.tensor_copy(stage_o[:], ot_ps[1][:])
                else:
                    nc.vector.tensor_copy(stage_e[:], ot_ps[0][:])
                    nc.vector.tensor_copy(stage_o[:], ot_ps[1][:])
                nc.vector.tensor_copy(pair_tile[0:64, :], stage_e[0:64, :])
                dq.dma_start(pair_tile[64:128, :], stage_o[0:64, :])
                dq.dma_start(den_c[2 * m:2 * m + 1, :], stage_e[64:65, :])
                dq.dma_start(den_c[2 * m + 1:2 * m + 2, :], stage_o[64:65, :])
                return stage_e

            def gen_epilogue(c, ot_sb, den_c, tail=False):
                """normalize (recip -> mask-matmul broadcast -> multiply) and
                the Wo projection for one n_q chunk. Generator: yields after
                each PE op so mid-stream epilogues interleave with the next
                unit's S matmuls instead of queueing ahead of them."""
                den_f = wk_pool.tile([4, 512], F32, tag="denf", name="denf", bufs=1)
                recip_f = wk_pool.tile([4, 512], F32, tag="recf", name="recf", bufs=1)
                recip_b = wk_pool.tile([4, 512], BF16, tag="recb", name="recb", bufs=1)
                nc.vector.tensor_copy(den_f[:], den_c[:])
                nc.vector.reciprocal_approx_fast(recip_f[:], den_f[:])
                nc.vector.tensor_copy(recip_b[:], recip_f[:])
                for m in range(2):
                    bc = psum.tile([128, 512], F32, tag="epi", name="bc", bufs=2)
                    nc.tensor.matmul(
                        bc[:], lhsT=e_mask[m], rhs=recip_b[:], start=True, stop=True,
                    )
                    yield
                    nc.vector.tensor_mul(ot_sb[m][:], ot_sb[m][:], bc[:])
                for s in range(4):
                    for dch in range(2):
                        # tail: sp banks are dead after the last exp --
                        # alternate tags to double the accumulate/copy
                        # pipeline depth of the final Wo chain
                        otag = "sp" if tail and (2 * s + dch) % 2 else "epi"
                        ops = psum.tile([128, 512], F32, tag=otag, name="op", bufs=2)
                        for m in range(2):
                            nc.tensor.matmul(
                                ops[:],
                                lhsT=ot_sb[m][:, 128 * s:128 * (s + 1)],
                                rhs=wo_sb[m][:, 512 * dch:512 * (dch + 1)],
                                start=(m == 0),
                                stop=(m == 1),
                            )
                            yield
                        o_sb = wk_pool.tile([128, 512], BF16, tag="osb", name="osb", bufs=4)
                        if tail and (s + dch) % 2 == 0:
                            nc.scalar.copy(o_sb[:], ops[:])
                        else:
                            nc.vector.tensor_copy(o_sb[:], ops[:])
                        r0 = 512 * c + 128 * s
                        nc.sync.dma_start(
                            out[r0:r0 + 128, 512 * dch:512 * (dch + 1)], o_sb[:]
                        )

            # ---- prologue: just enough projection for unit (0, 0)
            for _ in gen_kproj(0, 0):
                pass
            for _ in gen_qproj(0, 0):
                pass

            # ---- unit schedule: u = (pair m=u%2, chunk c=u//2), 2-deep
            # pipeline; feeders carry the remaining projections JIT.
            units = [(u % 2, u // 2) for u in range(2 * NQC)]
            feeders = {
                0: [gen_kproj(0, 1), gen_kproj(1, 0), gen_kproj(1, 1),
                    gen_qproj(1, 0)],
                1: [gen_vproj(j) for j in range(5)] + [gen_qproj(0, 1)]
                   + [gen_vproj(j) for j in range(5, 10)],
                2: [gen_vproj(j) for j in range(10, KT)] + [gen_qproj(1, 1)],
                3: [gen_qproj(0, 2), gen_qproj(1, 2)],
                4: [gen_qproj(0, 3), gen_qproj(1, 3)],
            }
            es_held = {}
            ot_ps_of = {}
            ot_sb_of = {}
            den_of = {}
            pending = []

            _DONE = object()

            def feeder_step(gens, k):
                done = 0
                while done < k and gens:
                    if next(gens[0], _DONE) is _DONE:
                        gens.pop(0)
                    else:
                        done += 1

            for u, (m, c) in enumerate(units):
                if u % 2 == 0:
                    den_of[c] = wk_pool.tile([4, 512], BF16, tag="den", name="den", bufs=2)
                ot_sb_of[u] = wk_pool.tile(
                    [128, 512], BF16, tag=f"ot{u % 4}", name=f"ot{u}", bufs=1
                )
                gens = pending + feeders.get(u, [])
                pending = []
                if u >= 2:
                    ot_ps_of[u - 2] = new_ot_ps()
                es_held[u] = []
                for j in range(KT):
                    es_held[u].append(emit_s_exp(m, c, j))
                    if u >= 2 and j >= 2:
                        up = u - 2
                        emit_o(units[up][0], j - 2, es_held[up][j - 2], ot_ps_of[up])
                    feeder_step(gens, 5)
                while gens:
                    feeder_step(gens, 16)
                if u >= 2:
                    up = u - 2
                    for j in (KT - 2, KT - 1):
                        emit_o(units[up][0], j, es_held[up][j], ot_ps_of[up])
                    mp, cp = units[up]
                    unload_pair(mp, ot_ps_of[up], ot_sb_of[up], den_of[cp])
                    del es_held[up]
                    if mp == 1:
                        pending.append(gen_epilogue(
                            cp, [ot_sb_of[2 * cp], ot_sb_of[2 * cp + 1]],
                            den_of[cp], tail=False,
                        ))

            # drain: both remaining O-passes interleaved back-to-back on
            # the PE (last unit's accumulators live in the sp banks, dead
            # after the final exp), with the chunk-2 epilogue woven in.
            def keep_warm(n, dep=None):
                # rhs reads the unload's staging tile so the scheduler can't
                # hoist these ahead of it -- they must bridge the PE hole
                rhs = wr[:] if dep is None else dep[0:64, :]
                lhs = wa[:] if dep is None else wa[0:64, :]
                for _ in range(n):
                    wps = psum.tile([16, 512], F32, tag="epi", name="kw", bufs=2)
                    nc.tensor.matmul(
                        wps[:], lhsT=lhs, rhs=rhs, start=True, stop=True
                    )

            u6, u7 = 2 * NQC - 2, 2 * NQC - 1
            mp6, cp6 = units[u6]
            mp7, cp7 = units[u7]
            ot_ps_of[u6] = new_ot_ps()
            ot_ps_of[u7] = [
                psum.tile([65, 512], F32, tag="sp", name=f"otL{h}", bufs=2)
                for h in range(2)
            ]
            gens = pending
            pending = []
            for j in range(KT):
                emit_o(mp6, j, es_held[u6][j], ot_ps_of[u6])
                emit_o(mp7, j, es_held[u7][j], ot_ps_of[u7])
                feeder_step(gens, 2)
            while gens:
                feeder_step(gens, 16)
            st6 = unload_pair(mp6, ot_ps_of[u6], ot_sb_of[u6], den_of[cp6],
                              tail=True, dq=nc.sync)
            keep_warm(4, dep=st6)
            st7 = unload_pair(mp7, ot_ps_of[u7], ot_sb_of[u7], den_of[cp7],
                              tail=True, dq=nc.gpsimd)
            keep_warm(4, dep=st7)
            for _ in gen_epilogue(
                NQC - 1, [ot_sb_of[2 * NQC - 2], ot_sb_of[2 * NQC - 1]],
                den_of[NQC - 1], tail=True,
            ):
                pass

            if DBG:
                nc.sync.dma_start(dbg["dkpA0"][:, :], kpA[0][:])
                nc.sync.dma_start(dbg["dqpA0"][:, :], qpA[0][:])
                nc.sync.dma_start(dbg["dvpa"][:, :], vpa[:])

    nc.compile()
    return nc


_NC_CACHE = None


def _get_nc():
    global _NC_CACHE
    if _NC_CACHE is None:
        _NC_CACHE = _build_nc()
    return _NC_CACHE


def make_in_maps(q, k, v, Wq, Wk, Wv, Wo):
    q = np.asarray(q, dtype=np.float32)
    k = np.asarray(k, dtype=np.float32)
    v = np.asarray(v, dtype=np.float32)
    qT = [np.ascontiguousarray(q[g].T).astype(NPBF16) for g in range(B)]
    kT = [np.ascontiguousarray(k[g].T).astype(NPBF16) for g in range(B)]
    vT = [np.ascontiguousarray(v[g].T).astype(NPBF16) for g in range(B)]
    wq_b = np.asarray(Wq, np.float32).astype(NPBF16)
    wk_b = np.asarray(Wk, np.float32).astype(NPBF16)
    wv_b = np.asarray(Wv, np.float32).astype(NPBF16)
    wo_b = np.asarray(Wo, np.float32).astype(NPBF16)
    emask = np.zeros((4, 256), NPBF16)
    for m in range(2):
        emask[2 * m, 128 * m:128 * m + 64] = 1
        emask[2 * m + 1, 128 * m + 64:128 * m + 128] = 1

    in_maps = []
    for c in range(8):
        g, t = c // 4, c % 4
        sl = slice(INNER * t, INNER * (t + 1))
        in_maps.append({
            "qT": qT[g],
            "kT": kT[g],
            "vT": vT[g],
            "wq": np.ascontiguousarray(wq_b[:, sl]),
            "wk": np.ascontiguousarray(wk_b[:, sl]),
            "wv": np.ascontiguousarray(wv_b[:, sl]),
            "wo": np.ascontiguousarray(wo_b[sl, :]),
            "emask": emask,
        })
    return in_maps


def kernel(q, k, v, Wq, Wk, Wv, Wo):
    in_maps = make_in_maps(q, k, v, Wq, Wk, Wv, Wo)
    nc = _get_nc()
    res = run_bass_kernel_spmd(nc, in_maps, core_ids=list(range(8)))

    out = np.empty((B, N, D), np.float32)
    for g in range(B):
        acc = res.results[4 * g]["out"].astype(np.float32)
        for t in range(1, 4):
            acc = acc + res.results[4 * g + t]["out"].astype(np.float32)
        out[g] = acc
    return out
